# revision 12
# baseline (speedup 1.0000x reference)
"""Squared euclidean distance kernel for Trainium2 (8 NeuronCores, SPMD).

dist[n, m] = ||mat_1[n]||^2 + ||mat_2[m]||^2 - 2 <mat_1[n], mat_2[m]>

Strategy: data-parallel shard of mat_1 rows across 8 cores; mat_2 replicated.
The whole computation is a single TensorE matmul per output tile with an
augmented contract dimension (K = 64 + 4):

    lhsT = [mat_1^T ; sq1_hi ; sq1_lo ; 1 ; 1]          (per core, [68, 12544] bf16)
    rhs  = [-2*mat_2^T ; 1 ; 1 ; sq2_hi ; sq2_lo]       (replicated, [68, 2048] bf16)

so PSUM accumulates the final distance in f32 directly (the squared norms are
carried as bf16 hi/lo pairs, recovering ~f32 accuracy for the norm terms).
The kernel is output-DMA bound: 103 MB of f32 distances per core.
"""

import numpy as np
import ml_dtypes

import concourse.bass as bass
import concourse.mybir as mybir
from concourse.tile import TileContext
from concourse.bass_utils import run_bass_kernel_spmd

N1, D, N2 = 100000, 64, 2048
NCORES = 8
ROWS_VALID = N1 // NCORES          # 12500 rows of mat_1 per core
CHUNK = 128                        # output rows per tile (PE partition dim)
NCHUNK = (ROWS_VALID + CHUNK - 1) // CHUNK   # 98
ROWS = CHUNK * NCHUNK              # 12544 (padded)
K = D + 4                          # 68: 64 features + sq1_hi/lo + ones
BANK = 512                         # fp32 PSUM bank width (max matmul free dim)
BF16 = ml_dtypes.bfloat16

_CACHE = {}


def _split_multi_waits(nc):
    """Walrus in this toolchain only accepts one sync-wait per instruction.
    Tile's add_semaphores can attach several (one per producer). Hoist all but
    one onto dedicated NoOps immediately before the instruction on the same
    engine stream — same semantics, each carrying a single wait."""
    for f in nc.m.functions:
        for bb in f.blocks:
            new = []
            for inst in bb.instructions:
                si = getattr(inst, "sync_info", None)
                if si is not None and si.on_wait is not None and len(si.on_wait) > 1:
                    for w in si.on_wait[:-1]:
                        nop = mybir.InstNoOp(
                            name=nc.get_next_instruction_name(), ins=[], outs=[]
                        )
                        nop.engine = inst.engine
                        nop.sync_info = mybir.SyncInfo(on_wait=[w], on_update=[])
                        new.append(nop)
                    si.on_wait = [si.on_wait[-1]]
                new.append(inst)
            bb.instructions[:] = new


def _build(nc, tc, lhst, rhs, out, rows, n2, out_bufs, lhs_splits, dma_chunks,
           dual_ring, loop_ctx=None):
    """Emit the pipeline (everything after dram tensor declarations).
    loop_ctx, if given, is a zero-arg callable returning a context manager
    that wraps the per-chunk loop (used for the timing For-loop)."""
    nchunk = rows // CHUNK
    nbank = n2 // BANK
    half = (nbank // 2) * BANK     # DVE copies [0:half), ACT copies [half:n2)

    with tc.tile_pool(name="const", bufs=1) as cpool, \
         tc.tile_pool(name="outp", bufs=out_bufs) as opool, \
         tc.tile_pool(name="psum", bufs=2, space="PSUM") as ppool:
        # Replicated rhs and the full per-core lhsT live in SBUF for the
        # whole kernel. lhsT is DMA'd in column-range pieces so early chunks
        # don't wait on the full 1.7 MB transfer. SWDGE (gpsimd) keeps the
        # HWDGE rings free for the output stream.
        rhs_sb = cpool.tile([K, n2], mybir.dt.bfloat16)
        nc.gpsimd.dma_start(out=rhs_sb[:], in_=rhs[:, :])

        lhs_sb = cpool.tile([K, rows], mybir.dt.bfloat16)
        split = max(CHUNK, rows // lhs_splits // CHUNK * CHUNK)
        for s0 in range(0, rows, split):
            s1 = min(s0 + split, rows)
            nc.gpsimd.dma_start(out=lhs_sb[:, s0:s1], in_=lhst[:, s0:s1])

        import contextlib
        ctx = loop_ctx() if loop_ctx is not None else contextlib.nullcontext()
        with ctx:
            for g0 in range(0, nchunk, dma_chunks):
                g = min(dma_chunks, nchunk - g0)
                ot = opool.tile([CHUNK, g * n2], mybir.dt.float32)
                for j in range(g):
                    c = g0 + j
                    ps = ppool.tile([CHUNK, n2], mybir.dt.float32)
                    w = lhs_sb[:, c * CHUNK:(c + 1) * CHUNK]
                    for b in range(nbank):
                        nc.tensor.matmul(
                            ps[:, b * BANK:(b + 1) * BANK],
                            w,
                            rhs_sb[:, b * BANK:(b + 1) * BANK],
                            start=True,
                            stop=True,
                        )
                    o = j * n2
                    if half > 0:
                        nc.vector.tensor_copy(
                            out=ot[:, o:o + half], in_=ps[:, :half]
                        )
                    if half < n2:
                        nc.scalar.copy(
                            out=ot[:, o + half:o + n2], in_=ps[:, half:]
                        )
                dram = out[g0 * CHUNK:(g0 + g) * CHUNK, :]
                src = ot[:]
                if g > 1:
                    dram = dram.rearrange("(j p) m -> p j m", p=CHUNK)
                    src = src.rearrange("p (j m) -> p j m", j=g)
                eng = nc.scalar if (dual_ring and (g0 // dma_chunks) % 2) else nc.sync
                eng.dma_start(out=dram, in_=src)


def build_nc(rows=ROWS, n2=N2, out_bufs=6, lhs_splits=8, dma_chunks=2,
             dual_ring=False):
    """Build the per-core Bass program (SPMD: same program on all 8 cores)."""
    nc = bass.Bass()
    lhst = nc.dram_tensor("lhst", [K, rows], mybir.dt.bfloat16, kind="ExternalInput")
    rhs = nc.dram_tensor("rhs", [K, n2], mybir.dt.bfloat16, kind="ExternalInput")
    out = nc.dram_tensor("out", [rows, n2], mybir.dt.float32, kind="ExternalOutput")

    with TileContext(nc) as tc:
        _build(nc, tc, lhst, rhs, out, rows, n2, out_bufs, lhs_splits,
               dma_chunks, dual_ring)

    _split_multi_waits(nc)
    return nc


def build_timing_nc(rows=ROWS, n2=N2, out_bufs=6, lhs_splits=8, dma_chunks=1,
                    dual_ring=False, repeats=8):
    """Same pipeline, repeated `repeats` times via a hardware For loop, with
    the big output going to internal DRAM scratch (no host transfer) and a
    tiny external output. Used only for wall-clock timing of HW exec."""
    nc = bass.Bass()
    lhst = nc.dram_tensor("lhst", [K, rows], mybir.dt.bfloat16, kind="ExternalInput")
    rhs = nc.dram_tensor("rhs", [K, n2], mybir.dt.bfloat16, kind="ExternalInput")
    out = nc.dram_tensor("scratch_out", [rows, n2], mybir.dt.float32,
                         kind="Internal")
    tout = nc.dram_tensor("tout", [1, 4], mybir.dt.float32,
                          kind="ExternalOutput")

    with TileContext(nc) as tc:
        _build(nc, tc, lhst, rhs, out, rows, n2, out_bufs, lhs_splits,
               dma_chunks, dual_ring, loop_ctx=lambda: tc.For_i(0, repeats, 1))

        with tc.tile_pool(name="tiny", bufs=1) as tpool:
            dt = tpool.tile([1, 4], mybir.dt.float32)
            nc.gpsimd.memset(dt[:], 0.0)
            nc.sync.dma_start(out=tout[:, :], in_=dt[:])

    _split_multi_waits(nc)
    return nc


def _prep_inputs(mat_1, mat_2, rows=ROWS, rows_valid=ROWS_VALID, n2=N2):
    """Host-side: shard + transpose + augment, f32 -> bf16 (hi/lo for norms)."""
    mat_1 = np.ascontiguousarray(np.asarray(mat_1, dtype=np.float32))
    mat_2 = np.ascontiguousarray(np.asarray(mat_2, dtype=np.float32))

    sq1 = np.square(mat_1, dtype=np.float32).sum(axis=1, dtype=np.float32)
    sq2 = np.square(mat_2, dtype=np.float32).sum(axis=1, dtype=np.float32)

    def hi_lo(v):
        hi = v.astype(BF16)
        lo = (v - hi.astype(np.float32)).astype(BF16)
        return hi, lo

    hi1, lo1 = hi_lo(sq1)
    hi2, lo2 = hi_lo(sq2)

    rhs = np.zeros((K, n2), dtype=BF16)
    rhs[0:D] = (-2.0 * mat_2.T).astype(BF16)
    rhs[D] = 1
    rhs[D + 1] = 1
    rhs[D + 2] = hi2
    rhs[D + 3] = lo2

    in_maps = []
    for c in range(NCORES):
        sl = slice(c * rows_valid, (c + 1) * rows_valid)
        lt = np.zeros((K, rows), dtype=BF16)
        lt[0:D, :rows_valid] = mat_1[sl].T.astype(BF16)
        lt[D, :rows_valid] = hi1[sl]
        lt[D + 1, :rows_valid] = lo1[sl]
        lt[D + 2] = 1
        lt[D + 3] = 1
        in_maps.append({"lhst": lt, "rhs": rhs})
    return in_maps


def kernel(mat_1, mat_2):
    if "nc" not in _CACHE:
        _CACHE["nc"] = build_nc()
    nc = _CACHE["nc"]
    in_maps = _prep_inputs(mat_1, mat_2)
    last_err = None
    for _ in range(3):
        try:
            res = run_bass_kernel_spmd(nc, in_maps, core_ids=list(range(NCORES)))
            break
        except Exception as e:  # rare transient NRT device errors
            last_err = e
    else:
        raise last_err
    return np.concatenate(
        [res.results[c]["out"][:ROWS_VALID] for c in range(NCORES)], axis=0
    )


# revision 20
# speedup vs baseline: 2.2672x; 2.2672x over previous
"""Squared euclidean distance kernel for Trainium2 (8 NeuronCores, SPMD).

dist[n, m] = ||mat_1[n]||^2 + ||mat_2[m]||^2 - 2 <mat_1[n], mat_2[m]>

Strategy: data-parallel shard of mat_1 rows across 8 cores; mat_2 replicated.
The whole computation is a single TensorE matmul per output tile with an
augmented contract dimension (K = 64 + 4):

    lhsT = [mat_1^T ; sq1_hi ; sq1_lo ; 1 ; 1]          (per core, [68, 12544] fp16)
    rhs  = [-2*mat_2^T ; 1 ; 1 ; sq2_hi ; sq2_lo]       (replicated, [68, 2048] fp16)

so PSUM accumulates the final distance in f32 directly (the squared norms are
carried as fp16 hi/lo pairs, recovering ~f32 accuracy for the norm terms;
fp16 runs at the same PE rate as bf16 here but with 4x finer mantissa).
The kernel is output-DMA bound: 103 MB of f32 distances per core at
~340 GB/s/core HBM write bandwidth -> ~300 us.
"""

import numpy as np
import ml_dtypes

import concourse.bass as bass
import concourse.mybir as mybir
from concourse.tile import TileContext
from concourse.bass_utils import run_bass_kernel_spmd

N1, D, N2 = 100000, 64, 2048
NCORES = 8
ROWS_VALID = N1 // NCORES          # 12500 rows of mat_1 per core
CHUNK = 128                        # output rows per tile (PE partition dim)
NCHUNK = (ROWS_VALID + CHUNK - 1) // CHUNK   # 98
ROWS = CHUNK * NCHUNK              # 12544 (padded)
K = D + 4                          # 68: 64 features + sq1_hi/lo + ones
BANK = 512                         # fp32 PSUM bank width (max matmul free dim)
BF16 = ml_dtypes.bfloat16

_CACHE = {}


def _split_multi_waits(nc):
    """Walrus in this toolchain only accepts one sync-wait per instruction.
    Tile's add_semaphores can attach several (one per producer). Hoist all but
    one onto dedicated NoOps immediately before the instruction on the same
    engine stream — same semantics, each carrying a single wait."""
    for f in nc.m.functions:
        for bb in f.blocks:
            new = []
            for inst in bb.instructions:
                si = getattr(inst, "sync_info", None)
                if si is not None and si.on_wait is not None and len(si.on_wait) > 1:
                    for w in si.on_wait[:-1]:
                        nop = mybir.InstNoOp(
                            name=nc.get_next_instruction_name(), ins=[], outs=[]
                        )
                        nop.engine = inst.engine
                        nop.sync_info = mybir.SyncInfo(on_wait=[w], on_update=[])
                        new.append(nop)
                    si.on_wait = [si.on_wait[-1]]
                new.append(inst)
            bb.instructions[:] = new


def _build(nc, tc, lhst, rhs, out, rows, n2, out_bufs, lhs_splits, dma_chunks,
           dual_ring, loop_ctx=None, dtype=mybir.dt.bfloat16,
           lhst_lo=None, rhs_lo=None):
    """Emit the pipeline (everything after dram tensor declarations).
    loop_ctx, if given, is a zero-arg callable returning a context manager
    that wraps the per-chunk loop (used for the timing For-loop)."""
    nchunk = rows // CHUNK
    nbank = n2 // BANK
    half = (nbank // 2) * BANK     # DVE copies [0:half), ACT copies [half:n2)

    with tc.tile_pool(name="const", bufs=1) as cpool, \
         tc.tile_pool(name="outp", bufs=out_bufs) as opool, \
         tc.tile_pool(name="psum", bufs=2, space="PSUM") as ppool:
        # Replicated rhs and the full per-core lhsT live in SBUF for the
        # whole kernel. lhsT is DMA'd in column-range pieces so early chunks
        # don't wait on the full 1.7 MB transfer. SWDGE (gpsimd) keeps the
        # HWDGE rings free for the output stream.
        rhs_sb = cpool.tile([K, n2], dtype)
        nc.gpsimd.dma_start(out=rhs_sb[:], in_=rhs[:, :])

        precise = lhst_lo is not None
        if precise:
            rhs_lo_sb = cpool.tile([D, n2], dtype)
            nc.gpsimd.dma_start(out=rhs_lo_sb[:], in_=rhs_lo[:, :])
            lhs_lo_sb = cpool.tile([D, rows], dtype)

        lhs_sb = cpool.tile([K, rows], dtype)
        split = max(CHUNK, rows // lhs_splits // CHUNK * CHUNK)
        for s0 in range(0, rows, split):
            s1 = min(s0 + split, rows)
            nc.gpsimd.dma_start(out=lhs_sb[:, s0:s1], in_=lhst[:, s0:s1])
            if precise:
                nc.gpsimd.dma_start(
                    out=lhs_lo_sb[:, s0:s1], in_=lhst_lo[:, s0:s1]
                )

        import contextlib
        ctx = loop_ctx() if loop_ctx is not None else contextlib.nullcontext()
        with ctx:
            for g0 in range(0, nchunk, dma_chunks):
                g = min(dma_chunks, nchunk - g0)
                ot = opool.tile([CHUNK, g * n2], mybir.dt.float32)
                for j in range(g):
                    c = g0 + j
                    ps = ppool.tile([CHUNK, n2], mybir.dt.float32)
                    w = lhs_sb[:, c * CHUNK:(c + 1) * CHUNK]
                    if precise:
                        w_hi = lhs_sb[:D, c * CHUNK:(c + 1) * CHUNK]
                        w_lo = lhs_lo_sb[:, c * CHUNK:(c + 1) * CHUNK]
                    for b in range(nbank):
                        sl = slice(b * BANK, (b + 1) * BANK)
                        nc.tensor.matmul(
                            ps[:, sl], w, rhs_sb[:, sl],
                            start=True, stop=not precise,
                        )
                        if precise:
                            nc.tensor.matmul(
                                ps[:, sl], w_hi, rhs_lo_sb[:, sl],
                                start=False, stop=False,
                            )
                            nc.tensor.matmul(
                                ps[:, sl], w_lo, rhs_sb[:D, sl],
                                start=False, stop=True,
                            )
                    o = j * n2
                    if half > 0:
                        nc.vector.tensor_copy(
                            out=ot[:, o:o + half], in_=ps[:, :half]
                        )
                    if half < n2:
                        nc.scalar.copy(
                            out=ot[:, o + half:o + n2], in_=ps[:, half:]
                        )
                dram = out[g0 * CHUNK:(g0 + g) * CHUNK, :]
                src = ot[:]
                if g > 1:
                    dram = dram.rearrange("(j p) m -> p j m", p=CHUNK)
                    src = src.rearrange("p (j m) -> p j m", j=g)
                eng = nc.scalar if (dual_ring and (g0 // dma_chunks) % 2) else nc.sync
                eng.dma_start(out=dram, in_=src)


def build_nc(rows=ROWS, n2=N2, out_bufs=6, lhs_splits=8, dma_chunks=2,
             dual_ring=False, dtype=mybir.dt.bfloat16, precise=False):
    """Build the per-core Bass program (SPMD: same program on all 8 cores)."""
    nc = bass.Bass()
    lhst = nc.dram_tensor("lhst", [K, rows], dtype, kind="ExternalInput")
    rhs = nc.dram_tensor("rhs", [K, n2], dtype, kind="ExternalInput")
    lhst_lo = rhs_lo = None
    if precise:
        lhst_lo = nc.dram_tensor("lhst_lo", [D, rows], dtype, kind="ExternalInput")
        rhs_lo = nc.dram_tensor("rhs_lo", [D, n2], dtype, kind="ExternalInput")
    out = nc.dram_tensor("out", [rows, n2], mybir.dt.float32, kind="ExternalOutput")

    with TileContext(nc) as tc:
        _build(nc, tc, lhst, rhs, out, rows, n2, out_bufs, lhs_splits,
               dma_chunks, dual_ring, dtype=dtype, lhst_lo=lhst_lo,
               rhs_lo=rhs_lo)

    _split_multi_waits(nc)
    return nc


def build_timing_nc(rows=ROWS, n2=N2, out_bufs=6, lhs_splits=8, dma_chunks=2,
                    dual_ring=False, repeats=8, dtype=mybir.dt.bfloat16,
                    precise=False):
    """Same pipeline, repeated `repeats` times via a hardware For loop, with
    the big output going to internal DRAM scratch (no host transfer) and a
    tiny external output. Used only for wall-clock timing of HW exec."""
    nc = bass.Bass()
    lhst = nc.dram_tensor("lhst", [K, rows], dtype, kind="ExternalInput")
    rhs = nc.dram_tensor("rhs", [K, n2], dtype, kind="ExternalInput")
    lhst_lo = rhs_lo = None
    if precise:
        lhst_lo = nc.dram_tensor("lhst_lo", [D, rows], dtype, kind="ExternalInput")
        rhs_lo = nc.dram_tensor("rhs_lo", [D, n2], dtype, kind="ExternalInput")
    out = nc.dram_tensor("scratch_out", [rows, n2], mybir.dt.float32,
                         kind="Internal")
    tout = nc.dram_tensor("tout", [1, 4], mybir.dt.float32,
                          kind="ExternalOutput")

    with TileContext(nc) as tc:
        _build(nc, tc, lhst, rhs, out, rows, n2, out_bufs, lhs_splits,
               dma_chunks, dual_ring, loop_ctx=lambda: tc.For_i(0, repeats, 1),
               dtype=dtype, lhst_lo=lhst_lo, rhs_lo=rhs_lo)

        with tc.tile_pool(name="tiny", bufs=1) as tpool:
            dt = tpool.tile([1, 4], mybir.dt.float32)
            nc.gpsimd.memset(dt[:], 0.0)
            nc.sync.dma_start(out=tout[:, :], in_=dt[:])

    _split_multi_waits(nc)
    return nc


def _prep_inputs(mat_1, mat_2, rows=ROWS, rows_valid=ROWS_VALID, n2=N2,
                 np_dtype=BF16, precise=False):
    """Host-side: shard + transpose + augment, f32 -> bf16 (hi/lo for norms).
    With np_dtype=float32 the hi/lo split degenerates to (v, 0) and the
    augmentation is exact."""
    mat_1 = np.ascontiguousarray(np.asarray(mat_1, dtype=np.float32))
    mat_2 = np.ascontiguousarray(np.asarray(mat_2, dtype=np.float32))

    sq1 = np.square(mat_1, dtype=np.float32).sum(axis=1, dtype=np.float32)
    sq2 = np.square(mat_2, dtype=np.float32).sum(axis=1, dtype=np.float32)

    def hi_lo(v):
        hi = v.astype(np_dtype)
        lo = (v - hi.astype(np.float32)).astype(np_dtype)
        return hi, lo

    hi1, lo1 = hi_lo(sq1)
    hi2, lo2 = hi_lo(sq2)

    neg2b = -2.0 * mat_2.T              # [D, n2] f32
    rhs = np.zeros((K, n2), dtype=np_dtype)
    rhs[0:D] = neg2b.astype(np_dtype)
    rhs[D] = 1
    rhs[D + 1] = 1
    rhs[D + 2] = hi2
    rhs[D + 3] = lo2
    if precise:
        rhs_lo = (neg2b - rhs[0:D].astype(np.float32)).astype(np_dtype)

    in_maps = []
    for c in range(NCORES):
        sl = slice(c * rows_valid, (c + 1) * rows_valid)
        m1t = mat_1[sl].T                # [D, rows_valid] f32
        lt = np.zeros((K, rows), dtype=np_dtype)
        lt[0:D, :rows_valid] = m1t.astype(np_dtype)
        lt[D, :rows_valid] = hi1[sl]
        lt[D + 1, :rows_valid] = lo1[sl]
        lt[D + 2] = 1
        lt[D + 3] = 1
        m = {"lhst": lt, "rhs": rhs}
        if precise:
            lt_lo = np.zeros((D, rows), dtype=np_dtype)
            lt_lo[:, :rows_valid] = (
                m1t - lt[0:D, :rows_valid].astype(np.float32)
            ).astype(np_dtype)
            m["lhst_lo"] = lt_lo
            m["rhs_lo"] = rhs_lo
        in_maps.append(m)
    return in_maps


def kernel(mat_1, mat_2):
    if "nc" not in _CACHE:
        _CACHE["nc"] = build_nc(dtype=mybir.dt.float16, precise=False)
    nc = _CACHE["nc"]
    in_maps = _prep_inputs(mat_1, mat_2, np_dtype=np.float16)
    last_err = None
    for _ in range(3):
        try:
            res = run_bass_kernel_spmd(nc, in_maps, core_ids=list(range(NCORES)))
            break
        except Exception as e:  # rare transient NRT device errors
            last_err = e
    else:
        raise last_err
    return np.concatenate(
        [res.results[c]["out"][:ROWS_VALID] for c in range(NCORES)], axis=0
    )


# revision 22
# speedup vs baseline: 2.3007x; 1.0148x over previous
"""Squared euclidean distance kernel for Trainium2 (8 NeuronCores, SPMD).

dist[n, m] = ||mat_1[n]||^2 + ||mat_2[m]||^2 - 2 <mat_1[n], mat_2[m]>

Strategy: data-parallel shard of mat_1 rows across 8 cores; mat_2 replicated.
The whole computation is a single TensorE matmul per output tile with an
augmented contract dimension (K = 64 + 4):

    lhsT = [mat_1^T ; sq1_hi ; sq1_lo ; 1 ; 1]          (per core, [68, 12544] fp16)
    rhs  = [-2*mat_2^T ; 1 ; 1 ; sq2_hi ; sq2_lo]       (replicated, [68, 2048] fp16)

so PSUM accumulates the final distance in f32 directly (the squared norms are
carried as fp16 hi/lo pairs, recovering ~f32 accuracy for the norm terms;
fp16 runs at the same PE rate as bf16 here but with 4x finer mantissa).
The kernel is output-DMA bound: 103 MB of f32 distances per core at
~340 GB/s/core HBM write bandwidth -> ~300 us.
"""

import numpy as np
import ml_dtypes

import concourse.bass as bass
import concourse.mybir as mybir
from concourse.tile import TileContext
from concourse.bass_utils import run_bass_kernel_spmd

N1, D, N2 = 100000, 64, 2048
NCORES = 8
ROWS_VALID = N1 // NCORES          # 12500 rows of mat_1 per core
CHUNK = 128                        # output rows per tile (PE partition dim)
NCHUNK = (ROWS_VALID + CHUNK - 1) // CHUNK   # 98
ROWS = CHUNK * NCHUNK              # 12544 (padded)
K = D + 4                          # 68: 64 features + sq1_hi/lo + ones
BANK = 512                         # fp32 PSUM bank width (max matmul free dim)
BF16 = ml_dtypes.bfloat16

_CACHE = {}


def _split_multi_waits(nc):
    """Walrus in this toolchain only accepts one sync-wait per instruction.
    Tile's add_semaphores can attach several (one per producer). Hoist all but
    one onto dedicated NoOps immediately before the instruction on the same
    engine stream — same semantics, each carrying a single wait."""
    for f in nc.m.functions:
        for bb in f.blocks:
            new = []
            for inst in bb.instructions:
                si = getattr(inst, "sync_info", None)
                if si is not None and si.on_wait is not None and len(si.on_wait) > 1:
                    for w in si.on_wait[:-1]:
                        nop = mybir.InstNoOp(
                            name=nc.get_next_instruction_name(), ins=[], outs=[]
                        )
                        nop.engine = inst.engine
                        nop.sync_info = mybir.SyncInfo(on_wait=[w], on_update=[])
                        new.append(nop)
                    si.on_wait = [si.on_wait[-1]]
                new.append(inst)
            bb.instructions[:] = new


def _build(nc, tc, lhst, rhs, out, rows, n2, out_bufs, lhs_splits, dma_chunks,
           dual_ring, loop_ctx=None, dtype=mybir.dt.bfloat16,
           lhst_lo=None, rhs_lo=None):
    """Emit the pipeline (everything after dram tensor declarations).
    loop_ctx, if given, is a zero-arg callable returning a context manager
    that wraps the per-chunk loop (used for the timing For-loop)."""
    nchunk = rows // CHUNK
    nbank = n2 // BANK
    half = (nbank // 2) * BANK     # DVE copies [0:half), ACT copies [half:n2)

    with tc.tile_pool(name="const", bufs=1) as cpool, \
         tc.tile_pool(name="outp", bufs=out_bufs) as opool, \
         tc.tile_pool(name="psum", bufs=2, space="PSUM") as ppool:
        # Replicated rhs and the full per-core lhsT live in SBUF for the
        # whole kernel. lhsT is DMA'd in column-range pieces so early chunks
        # don't wait on the full 1.7 MB transfer. SWDGE (gpsimd) keeps the
        # HWDGE rings free for the output stream.
        rhs_sb = cpool.tile([K, n2], dtype)
        nc.gpsimd.dma_start(out=rhs_sb[:], in_=rhs[:, :])

        precise = lhst_lo is not None
        if precise:
            rhs_lo_sb = cpool.tile([D, n2], dtype)
            nc.gpsimd.dma_start(out=rhs_lo_sb[:], in_=rhs_lo[:, :])
            lhs_lo_sb = cpool.tile([D, rows], dtype)

        lhs_sb = cpool.tile([K, rows], dtype)
        split = max(CHUNK, rows // lhs_splits // CHUNK * CHUNK)
        for s0 in range(0, rows, split):
            s1 = min(s0 + split, rows)
            nc.gpsimd.dma_start(out=lhs_sb[:, s0:s1], in_=lhst[:, s0:s1])
            if precise:
                nc.gpsimd.dma_start(
                    out=lhs_lo_sb[:, s0:s1], in_=lhst_lo[:, s0:s1]
                )

        import contextlib
        ctx = loop_ctx() if loop_ctx is not None else contextlib.nullcontext()
        with ctx:
            for g0 in range(0, nchunk, dma_chunks):
                g = min(dma_chunks, nchunk - g0)
                ot = opool.tile([CHUNK, g * n2], mybir.dt.float32)
                for j in range(g):
                    c = g0 + j
                    ps = ppool.tile([CHUNK, n2], mybir.dt.float32)
                    w = lhs_sb[:, c * CHUNK:(c + 1) * CHUNK]
                    if precise:
                        w_hi = lhs_sb[:D, c * CHUNK:(c + 1) * CHUNK]
                        w_lo = lhs_lo_sb[:, c * CHUNK:(c + 1) * CHUNK]
                    for b in range(nbank):
                        sl = slice(b * BANK, (b + 1) * BANK)
                        nc.tensor.matmul(
                            ps[:, sl], w, rhs_sb[:, sl],
                            start=True, stop=not precise,
                        )
                        if precise:
                            nc.tensor.matmul(
                                ps[:, sl], w_hi, rhs_lo_sb[:, sl],
                                start=False, stop=False,
                            )
                            nc.tensor.matmul(
                                ps[:, sl], w_lo, rhs_sb[:D, sl],
                                start=False, stop=True,
                            )
                    o = j * n2
                    if half > 0:
                        nc.vector.tensor_copy(
                            out=ot[:, o:o + half], in_=ps[:, :half]
                        )
                    if half < n2:
                        nc.scalar.copy(
                            out=ot[:, o + half:o + n2], in_=ps[:, half:]
                        )
                dram = out[g0 * CHUNK:(g0 + g) * CHUNK, :]
                src = ot[:]
                if g > 1:
                    dram = dram.rearrange("(j p) m -> p j m", p=CHUNK)
                    src = src.rearrange("p (j m) -> p j m", j=g)
                eng = nc.scalar if (dual_ring and (g0 // dma_chunks) % 2) else nc.sync
                eng.dma_start(out=dram, in_=src)


def build_nc(rows=ROWS, n2=N2, out_bufs=6, lhs_splits=8, dma_chunks=2,
             dual_ring=False, dtype=mybir.dt.bfloat16, precise=False):
    """Build the per-core Bass program (SPMD: same program on all 8 cores)."""
    nc = bass.Bass()
    lhst = nc.dram_tensor("lhst", [K, rows], dtype, kind="ExternalInput")
    rhs = nc.dram_tensor("rhs", [K, n2], dtype, kind="ExternalInput")
    lhst_lo = rhs_lo = None
    if precise:
        lhst_lo = nc.dram_tensor("lhst_lo", [D, rows], dtype, kind="ExternalInput")
        rhs_lo = nc.dram_tensor("rhs_lo", [D, n2], dtype, kind="ExternalInput")
    out = nc.dram_tensor("out", [rows, n2], mybir.dt.float32, kind="ExternalOutput")

    with TileContext(nc) as tc:
        _build(nc, tc, lhst, rhs, out, rows, n2, out_bufs, lhs_splits,
               dma_chunks, dual_ring, dtype=dtype, lhst_lo=lhst_lo,
               rhs_lo=rhs_lo)

    _split_multi_waits(nc)
    return nc


def build_timing_nc(rows=ROWS, n2=N2, out_bufs=6, lhs_splits=8, dma_chunks=2,
                    dual_ring=False, repeats=8, dtype=mybir.dt.bfloat16,
                    precise=False):
    """Same pipeline, repeated `repeats` times via a hardware For loop, with
    the big output going to internal DRAM scratch (no host transfer) and a
    tiny external output. Used only for wall-clock timing of HW exec."""
    nc = bass.Bass()
    lhst = nc.dram_tensor("lhst", [K, rows], dtype, kind="ExternalInput")
    rhs = nc.dram_tensor("rhs", [K, n2], dtype, kind="ExternalInput")
    lhst_lo = rhs_lo = None
    if precise:
        lhst_lo = nc.dram_tensor("lhst_lo", [D, rows], dtype, kind="ExternalInput")
        rhs_lo = nc.dram_tensor("rhs_lo", [D, n2], dtype, kind="ExternalInput")
    out = nc.dram_tensor("scratch_out", [rows, n2], mybir.dt.float32,
                         kind="Internal")
    tout = nc.dram_tensor("tout", [1, 4], mybir.dt.float32,
                          kind="ExternalOutput")

    with TileContext(nc) as tc:
        _build(nc, tc, lhst, rhs, out, rows, n2, out_bufs, lhs_splits,
               dma_chunks, dual_ring, loop_ctx=lambda: tc.For_i(0, repeats, 1),
               dtype=dtype, lhst_lo=lhst_lo, rhs_lo=rhs_lo)

        with tc.tile_pool(name="tiny", bufs=1) as tpool:
            dt = tpool.tile([1, 4], mybir.dt.float32)
            nc.gpsimd.memset(dt[:], 0.0)
            nc.sync.dma_start(out=tout[:, :], in_=dt[:])

    _split_multi_waits(nc)
    return nc


def _prep_inputs(mat_1, mat_2, rows=ROWS, rows_valid=ROWS_VALID, n2=N2,
                 np_dtype=BF16, precise=False):
    """Host-side: shard + transpose + augment, f32 -> np_dtype (hi/lo for
    norms). With np_dtype=float32 the hi/lo split degenerates to (v, 0) and
    the augmentation is exact."""
    mat_1 = np.ascontiguousarray(np.asarray(mat_1, dtype=np.float32))
    mat_2 = np.ascontiguousarray(np.asarray(mat_2, dtype=np.float32))

    sq1 = np.square(mat_1, dtype=np.float32).sum(axis=1, dtype=np.float32)
    sq2 = np.square(mat_2, dtype=np.float32).sum(axis=1, dtype=np.float32)

    def hi_lo(v):
        hi = v.astype(np_dtype)
        lo = (v - hi.astype(np.float32)).astype(np_dtype)
        return hi, lo

    hi1, lo1 = hi_lo(sq1)
    hi2, lo2 = hi_lo(sq2)

    neg2b = -2.0 * mat_2.T              # [D, n2] f32
    rhs = np.zeros((K, n2), dtype=np_dtype)
    rhs[0:D] = neg2b.astype(np_dtype)
    rhs[D] = 1
    rhs[D + 1] = 1
    rhs[D + 2] = hi2
    rhs[D + 3] = lo2
    if precise:
        rhs_lo = (neg2b - rhs[0:D].astype(np.float32)).astype(np_dtype)

    in_maps = []
    for c in range(NCORES):
        sl = slice(c * rows_valid, (c + 1) * rows_valid)
        m1t = mat_1[sl].T                # [D, rows_valid] f32
        lt = np.zeros((K, rows), dtype=np_dtype)
        lt[0:D, :rows_valid] = m1t.astype(np_dtype)
        lt[D, :rows_valid] = hi1[sl]
        lt[D + 1, :rows_valid] = lo1[sl]
        lt[D + 2] = 1
        lt[D + 3] = 1
        m = {"lhst": lt, "rhs": rhs}
        if precise:
            lt_lo = np.zeros((D, rows), dtype=np_dtype)
            lt_lo[:, :rows_valid] = (
                m1t - lt[0:D, :rows_valid].astype(np.float32)
            ).astype(np_dtype)
            m["lhst_lo"] = lt_lo
            m["rhs_lo"] = rhs_lo
        in_maps.append(m)
    return in_maps


def kernel(mat_1, mat_2):
    if "nc" not in _CACHE:
        _CACHE["nc"] = build_nc(dtype=mybir.dt.float16, precise=False,
                                dma_chunks=1, dual_ring=True)
    nc = _CACHE["nc"]
    in_maps = _prep_inputs(mat_1, mat_2, np_dtype=np.float16)
    last_err = None
    for _ in range(3):
        try:
            res = run_bass_kernel_spmd(nc, in_maps, core_ids=list(range(NCORES)))
            break
        except Exception as e:  # rare transient NRT device errors
            last_err = e
    else:
        raise last_err
    return np.concatenate(
        [res.results[c]["out"][:ROWS_VALID] for c in range(NCORES)], axis=0
    )
